# revision 29
# baseline (speedup 1.0000x reference)
"""AdvancedGCN (3-layer GCNConv + global_mean_pool + linear) on 8 Trainium2
NeuronCores via Bass/Tile.

Strategy (per 8-way node sharding of dst nodes):
  - GCN layer out[d] = dinv[d] * sum_{e: dst=d} (h[src]@W)*dinv[src] + b
    with self-loops folded in as explicit (n->n) edges.
  - Per layer: each core owns a 12.5k-node shard of dst nodes. The projected,
    dinv-prescaled feature table (N x 64, f32) lives in HBM, AllGathered
    across cores each layer.
  - Edge messages are fetched with dma_gather (one 256B descriptor per edge,
    int16 indices; the 100k-row table is addressed via 4 "residue" views of
    stride 1024B so indices fit int16).
  - The segment-sum over dst is a PE matmul: per 128-edge chunk, a one-hot
    [128 edges x 128 dst] matrix (built on DVE via iota==dst_local compare)
    scatter-adds messages into a PSUM accumulator per 128-dst block.
  - Graph mean-pool is another one-hot matmul; partial sums are AllReduced.

Host-side work is limited to integer index bookkeeping: bucketing edges by
(dst-block, src-residue), padding to 128-slot chunks, degree/graph counts
(np.bincount). All float math runs on device.
"""
import os
import sys
import types

sys.path.insert(0, "/opt/trn_rl_repo")

import numpy as np
import ml_dtypes

BF = ml_dtypes.bfloat16


def _install_ntff_hook():
    """The agent image's antenv lacks axon_hooks; fabricate it so
    run_bass_kernel_spmd(trace=True) can capture NTFF profiles."""
    try:
        import antenv
    except ImportError:
        return
    if "antenv.axon_hooks" in sys.modules:
        return
    mod = types.ModuleType("antenv.axon_hooks")
    mod._hook = None

    def set_axon_ntff_profile_hook(h):
        mod._hook = h

    def get_axon_ntff_profile_hook():
        return mod._hook

    mod.set_axon_ntff_profile_hook = set_axon_ntff_profile_hook
    mod.get_axon_ntff_profile_hook = get_axon_ntff_profile_hook
    sys.modules["antenv.axon_hooks"] = mod
    antenv.axon_hooks = mod
    try:
        from trn_agent_boot.trn_boot import _ntff_profile_via_ctypes

        hook = _ntff_profile_via_ctypes("/opt/axon/libaxon_pjrt.so")
        if hook is not None:
            mod._hook = hook
    except Exception:
        pass


_install_ntff_hook()

import concourse.bass as bass
import concourse.bacc as bacc
import concourse.mybir as mybir
import concourse.tile as tile
from concourse._compat import cdiv
from concourse.library_config import mlp
from concourse.masks import make_identity

F32 = mybir.dt.float32
BF16 = mybir.dt.bfloat16
I16 = mybir.dt.int16
AF = mybir.ActivationFunctionType
OP = mybir.AluOpType

CFG_FULL = dict(
    n_nodes=100000,
    n_graphs=256,
    d_in=128,
    hid=64,
    n_cls=10,
    n_cores=8,
    sw=2,  # dst blocks per gather super-window
)

R = 4  # src residues (table addressed as [N/4, 4*hid] so idx fits int16)
SINGLE_PACKET = False  # >64 descriptors per lane overflows a single packet


# --------------------------------------------------------------------------
# Host preprocessing: integer bucketing / template construction
# --------------------------------------------------------------------------

class T:
    """Template: program-shape constants + per-core input arrays."""


def preprocess(x, edge_index, batch, cfg):
    t = T()
    NC = cfg["n_cores"]
    N = cfg["n_nodes"]
    HID = cfg["hid"]
    NG = cfg["n_graphs"]
    assert N % NC == 0
    NPC = N // NC
    BLOCKS = cdiv(NPC, 128)
    SH = BLOCKS * 128
    NTOT = NC * SH
    assert NTOT % R == 0 and NTOT // R <= 32768

    t.cfg = cfg
    t.NPC, t.BLOCKS, t.SH, t.NTOT = NPC, BLOCKS, SH, NTOT
    t.GB = cdiv(NG, 128)

    src = edge_index[0].astype(np.int64)
    dst = edge_index[1].astype(np.int64)
    deg = (np.bincount(dst, minlength=N) + 1).astype(np.float32)

    S = src
    D = dst
    gsrc = (S // NPC) * SH + (S % NPC)  # padded global table row of src
    core = D // NPC
    dloc = D % NPC
    blk = dloc // 128
    dl = (dloc % 128).astype(np.float32)
    res = gsrc % R
    qidx = (gsrc // R).astype(np.int16)

    ncells = NC * BLOCKS * R
    key = ((core * BLOCKS + blk) * R + res).astype(np.int64)
    order = np.argsort(key, kind="stable")
    q_sorted = qidx[order]
    dl_sorted = dl[order]
    counts = np.bincount(key, minlength=ncells).reshape(NC, BLOCKS, R)
    starts = np.zeros(ncells + 1, dtype=np.int64)
    np.cumsum(counts.reshape(-1), out=starts[1:])

    chunks = np.ceil(counts.max(axis=0) / 128).astype(np.int64)  # [BLOCKS, R]
    slots = chunks * 128
    t.chunks = chunks

    # super-windows
    SW = cfg["sw"]
    t.sws = [list(range(i, min(i + SW, BLOCKS))) for i in range(0, BLOCKS, SW)]

    # gather-call sizes and idx column offsets (order: si asc, r asc)
    t.Lsr = [[int(slots[sw, r].sum()) for r in range(R)] for sw in t.sws]
    icol = []
    off = 0
    for si in range(len(t.sws)):
        icol.append([])
        for r in range(R):
            icol[si].append(off)
            off += t.Lsr[si][r] // 16
    t.icol, t.TOTC = icol, off

    # dlocal instance column offsets (order: b asc, r asc, j asc)
    dcol = np.zeros((BLOCKS, R), dtype=np.int64)
    off = 0
    for b in range(BLOCKS):
        for r in range(R):
            dcol[b, r] = off
            off += chunks[b, r]
    t.dcol, t.TOTI = dcol, int(off)

    # msgs column base of (b, r) within its super-window's residue-r buffer
    mcol = np.zeros((BLOCKS, R), dtype=np.int64)
    for sw in t.sws:
        for r in range(R):
            off = 0
            for b in sw:
                mcol[b, r] = off
                off += chunks[b, r]
    t.mcol = mcol

    # ---- per-core arrays ----
    xT = x.T.astype(np.float32)  # [d_in, N]
    t.per_core = []
    for c in range(NC):
        qpad = {}
        dlp = {}
        for b in range(BLOCKS):
            for r in range(R):
                m = (c * BLOCKS + b) * R + r
                s, e = starts[m], starts[m + 1]
                L = int(slots[b, r])
                qq = np.zeros(L, dtype=np.int16)
                dd = np.full(L, -1.0, dtype=np.float32)
                qq[: e - s] = q_sorted[s:e]
                dd[: e - s] = dl_sorted[s:e]
                qpad[(b, r)] = qq
                dlp[(b, r)] = dd

        gidx = np.zeros((128, t.TOTC), dtype=np.int16)
        for si, sw in enumerate(t.sws):
            for r in range(R):
                v = np.concatenate([qpad[(b, r)] for b in sw])
                w = v.reshape(-1, 16).T  # [16, L/16]; pos i -> (i%16, i//16)
                gidx[:, icol[si][r] : icol[si][r] + v.size // 16] = np.tile(w, (8, 1))

        # host-built one-hot scatter matrices: [TOTI, 128 slot, 128 dst] bf16,
        # laid out as [128 slot-partitions, TOTI*128] for DMA into SBUF.
        ohm = np.zeros((t.TOTI, 128, 128), dtype=BF)
        for b in range(BLOCKS):
            for r in range(R):
                dd = dlp[(b, r)].astype(np.int64)  # [slots]; -1 = pad
                for j in range(int(chunks[b, r])):
                    seg = dd[j * 128 : (j + 1) * 128]
                    val = seg >= 0
                    ohm[dcol[b, r] + j][np.nonzero(val)[0], seg[val]] = 1
        ohm = np.ascontiguousarray(ohm.swapaxes(0, 1).reshape(128, t.TOTI * 128))

        lo, hi = c * NPC, (c + 1) * NPC
        xTs = np.zeros((cfg["d_in"], SH), dtype=np.float32)
        xTs[:, :NPC] = xT[:, lo:hi]

        degs = np.ones(SH, dtype=np.float32)
        degs[:NPC] = deg[lo:hi]
        degw = degs.reshape(BLOCKS, 128).T.copy()  # [128, BLOCKS]

        bats = np.full(SH, -1, dtype=np.int64)
        bats[:NPC] = batch[lo:hi]
        batw = bats.reshape(BLOCKS, 128).T  # [128, BLOCKS]
        # host pool one-hots: [128, (b*GB+gb)*128 + c] = (batch == gb*128+c)
        ohg = np.zeros((128, BLOCKS * t.GB * 128), dtype=BF)
        for b in range(BLOCKS):
            for gb in range(t.GB):
                eq = batw[:, b : b + 1] == (gb * 128 + np.arange(128))[None, :]
                ohg[:, (b * t.GB + gb) * 128 : (b * t.GB + gb + 1) * 128] = eq
        t.per_core.append(dict(gidx=gidx, oh=ohm, ohg=ohg, xT=xTs, deg=degw))

    cnt = np.bincount(batch.astype(np.int64), minlength=NG).astype(np.float32)
    cntw = np.zeros((128, t.GB), dtype=np.float32)
    for gb in range(t.GB):
        n = min(128, NG - gb * 128)
        cntw[:n, gb] = cnt[gb * 128 : gb * 128 + n]
    t.cnt = cntw

    return t


def make_in_maps(t, W1, b1, W2, b2, W3, b3, Wlin, blin):
    cfg = t.cfg
    HID = cfg["hid"]
    shared = dict(
        cnt=t.cnt,
        w1=W1.astype(np.float32),
        w2=W2.astype(np.float32),
        w3=W3.astype(np.float32),
        wl=Wlin.astype(np.float32),
        b1t=np.tile(b1.astype(np.float32), (128, 1)),
        b2t=np.tile(b2.astype(np.float32), (128, 1)),
        b3t=np.tile(b3.astype(np.float32), (128, 1)),
        blt=np.tile(blin.astype(np.float32), (128, 1)),
    )
    return [dict(shared, **pc) for pc in t.per_core]


# --------------------------------------------------------------------------
# Device program
# --------------------------------------------------------------------------

def build_program(t, enable_asserts=False):
    cfg = t.cfg
    NC = cfg["n_cores"]
    HID = cfg["hid"]
    DIN = cfg["d_in"]
    NG = cfg["n_graphs"]
    NCLS = cfg["n_cls"]
    BLOCKS, SH, NTOT, GB = t.BLOCKS, t.SH, t.NTOT, t.GB
    chunks, Lsr, icol, dcol, mcol = t.chunks, t.Lsr, t.icol, t.dcol, t.mcol
    IW = max(256, 128 * GB)

    nc = bacc.Bacc(
        "TRN2",
        target_bir_lowering=False,
        debug=False,
        enable_asserts=enable_asserts,
        num_devices=NC,
        num_swdge_queues=4,
    )

    din = lambda n, s, d=F32: nc.dram_tensor(n, s, d, kind="ExternalInput")
    xT_d = din("xT", [DIN, SH])
    gidx_d = din("gidx", [128, t.TOTC], I16)
    ohd = din("oh", [128, t.TOTI * 128], BF16)
    deg_d = din("deg", [128, BLOCKS])
    ohg_d = din("ohg", [128, BLOCKS * GB * 128], BF16)
    cnt_d = din("cnt", [128, GB])
    w1_d = din("w1", [DIN, HID])
    w2_d = din("w2", [HID, HID])
    w3_d = din("w3", [HID, HID])
    wl_d = din("wl", [HID, NCLS])
    b1t_d = din("b1t", [128, HID])
    b2t_d = din("b2t", [128, HID])
    b3t_d = din("b3t", [128, HID])
    blt_d = din("blt", [128, NCLS])
    out_d = nc.dram_tensor("out", [NG, NCLS], F32, kind="ExternalOutput")

    tab = [nc.dram_tensor(f"table{k}", [NTOT, HID], BF16, addr_space="Shared") for k in range(3)]
    bnc = [nc.dram_tensor(f"bounce{k}", [SH, HID], BF16) for k in range(3)]
    pool_loc = nc.dram_tensor("pool_loc", [128 * GB, HID], F32)
    pool_sum = nc.dram_tensor("pool_sum", [128 * GB, HID], F32, addr_space="Shared")

    groups = [list(range(NC))]

    with tile.TileContext(nc) as tc:
        with (
            tc.tile_pool(name="const", bufs=1) as cp,
            tc.tile_pool(name="xw", bufs=3) as xp,
            tc.tile_pool(name="ix", bufs=2) as ixp,
            tc.tile_pool(name="msg", bufs=2) as mp,
            tc.tile_pool(name="oh", bufs=2) as ohp,
            tc.tile_pool(name="hall", bufs=1) as hap,
            tc.tile_pool(name="ep", bufs=3) as ep,
            tc.tile_pool(name="psb", bufs=2, space="PSUM") as psb,
            tc.tile_pool(name="pst", bufs=2, space="PSUM") as pst,
            tc.tile_pool(name="psw", bufs=2, space="PSUM") as psw,
            tc.tile_pool(name="psg", bufs=1, space="PSUM") as psg,
        ):
            nc.gpsimd.load_library(mlp)

            # ---- constants ----
            cnt_t = cp.tile([128, GB], F32, tag="cnt")
            nc.sync.dma_start(cnt_t[:], cnt_d[:, :])
            deg_t = cp.tile([128, BLOCKS], F32, tag="deg")
            nc.sync.dma_start(deg_t[:], deg_d[:, :])
            dsq_t = cp.tile([128, BLOCKS], F32, tag="dsq")
            nc.scalar.activation(dsq_t[:], deg_t[:], AF.Sqrt)
            dinv_t = cp.tile([128, BLOCKS], F32, tag="dinv")
            nc.vector.reciprocal(dinv_t[:], dsq_t[:])
            w1_t = cp.tile([DIN, HID], F32, tag="w1")
            nc.sync.dma_start(w1_t[:], w1_d[:, :])
            w2_t = cp.tile([HID, HID], F32, tag="w2")
            nc.sync.dma_start(w2_t[:], w2_d[:, :])
            w3_t = cp.tile([HID, HID], F32, tag="w3")
            nc.sync.dma_start(w3_t[:], w3_d[:, :])
            wl_t = cp.tile([HID, NCLS], F32, tag="wl")
            nc.sync.dma_start(wl_t[:], wl_d[:, :])
            bt = []
            for nm, d in (("b1t", b1t_d), ("b2t", b2t_d), ("b3t", b3t_d)):
                b_ = cp.tile([128, HID], F32, tag=nm)
                nc.sync.dma_start(b_[:], d[:, :])
                bt.append(b_)
            blt_t = cp.tile([128, NCLS], F32, tag="blt")
            nc.sync.dma_start(blt_t[:], blt_d[:, :])
            ident = cp.tile([128, 128], F32, tag="ident")
            make_identity(nc, ident[:])

            wnext = [w2_t, w3_t]

            # ---- phase 1: table0 = (x @ W1) * dinv ----
            with nc.named_scope("p1"):
                for b in range(BLOCKS):
                    xt = xp.tile([DIN, 128], F32, tag="xt")
                    nc.sync.dma_start(xt[:], xT_d[:, b * 128 : (b + 1) * 128])
                    ps = psw.tile([128, HID], F32, tag="psw")
                    nc.tensor.matmul(ps[:], lhsT=xt[:], rhs=w1_t[:], start=True, stop=True)
                    tb = ep.tile([128, HID], BF16, tag="tb")
                    nc.scalar.activation(tb[:], ps[:], AF.Copy, scale=dinv_t[:, b : b + 1])
                    nc.sync.dma_start(bnc[0][b * 128 : (b + 1) * 128, :], tb[:])
            with nc.named_scope("ag0"):
                nc.gpsimd.collective_compute(
                    "AllGather", OP.bypass, replica_groups=groups,
                    ins=[bnc[0].ap().opt()], outs=[tab[0].ap().opt()],
                )

            # ---- layers ----
            pool_ps = None
            for k in range(3):
                # pair views of the bf16 gather table: each 256B descriptor
                # fetches 2 adjacent rows; class r = 2*rp + h, rp picks the
                # pair within a row-quad, h the row within the pair.
                tview = tab[k].ap().rearrange("(a b) d -> a (b d)", b=R)
                rviews = [tview[:, (r // 2) * 2 * HID : (r // 2 + 1) * 2 * HID] for r in range(R)]
                if k == 2:
                    pool_ps = []
                    for gb in range(GB):
                        pps = psg.tile([128, HID], F32, tag=f"psg{gb}")
                        pool_ps.append(pps)

                sid, _ = nc.enter_named_scope(f"L{k}", notify=False)
                if k < 2:
                    hall = hap.tile([128, BLOCKS * HID], F32, tag="hall")
                else:
                    h3all = hap.tile([128, BLOCKS * HID], BF16, tag="h3all")
                for si, sw in enumerate(t.sws):
                    icw = sum(Lsr[si][r] // 16 for r in range(R))
                    ixt = ixp.tile([128, max(ix_max(t), 16)], I16, tag="ixt")
                    nc.sync.dma_start(ixt[:, :icw], gidx_d[:, icol[si][0] : icol[si][0] + icw])
                    dw = int(chunks[sw, :].sum())
                    ohl = ohp.tile([128, max(dl_max(t), 1) * 128], BF16, tag="ohl")
                    d0 = int(dcol[sw[0], 0])
                    nc.sync.dma_start(ohl[:, : dw * 128], ohd[:, d0 * 128 : (d0 + dw) * 128])

                    gts = []
                    for r in range(R):
                        L = Lsr[si][r]
                        cols = L // 128
                        gt = mp.tile([128, max(m_max(t), 1), 2 * HID], BF16, tag=f"m{r}")
                        if L:
                            a0 = icol[si][r] - icol[si][0]
                            nc.gpsimd.dma_gather(
                                gt[:, :cols, :], rviews[r], ixt[:, a0 : a0 + L // 16],
                                L, L, 2 * HID, elem_step=R * HID,
                                single_packet=SINGLE_PACKET,
                                queue_num=1 + (r + si) % 3,
                            )
                        gts.append(gt)

                    for b in sw:
                        ps = psb.tile([128, HID], F32, tag="psb")
                        nch = int(chunks[b, :].sum())
                        done = 0
                        for r in range(R):
                            ch = int(chunks[b, r])
                            if ch == 0:
                                continue
                            c0 = int(dcol[b, r]) - int(dcol[sw[0], 0])
                            hoff = (r % 2) * HID
                            for j in range(ch):
                                nc.tensor.matmul(
                                    ps[:],
                                    lhsT=ohl[:, (c0 + j) * 128 : (c0 + j + 1) * 128],
                                    rhs=gts[r][:, mcol[b, r] + j, hoff : hoff + HID],
                                    start=(done == 0),
                                    stop=(done == nch - 1),
                                )
                                done += 1

                        # ---- block epilogue: h = dinv*ps + (dinv*town + bias) ----
                        townb = ep.tile([128, HID], BF16, tag="townb")
                        nc.sync.dma_start(townb[:], bnc[k][b * 128 : (b + 1) * 128, :])
                        pre = ep.tile([128, HID], F32, tag="pre")
                        nc.scalar.activation(pre[:], townb[:], AF.Copy, scale=dinv_t[:, b : b + 1])
                        pre2 = ep.tile([128, HID], F32, tag="pre2")
                        nc.vector.tensor_add(pre2[:], pre[:], bt[k][:])
                        t1 = ep.tile([128, HID], F32, tag="t1")
                        nc.scalar.activation(t1[:], ps[:], AF.Copy, scale=dinv_t[:, b : b + 1])
                        hsl = slice(b * HID, (b + 1) * HID)
                        hp = ep.tile([128, HID], F32, tag="hp")
                        nc.vector.tensor_add(hp[:], t1[:], pre2[:])
                        if k < 2:
                            nc.scalar.activation(hall[:, hsl], hp[:], AF.Relu)
                        else:
                            nc.scalar.copy(h3all[:, hsl], hp[:])

                # ---- deferred PE phase: next-layer projection / pooling ----
                if k < 2:
                    for b in range(BLOCKS):
                        pt = pst.tile([128, 128], F32, tag="pst")
                        nc.tensor.transpose(pt[:HID, :], hall[:, b * HID : (b + 1) * HID], ident[:])
                        hT = ep.tile([HID, 128], F32, tag="hT")
                        nc.scalar.copy(hT[:], pt[:HID, :])
                        ps2 = psw.tile([128, HID], F32, tag="psw")
                        nc.tensor.matmul(ps2[:], lhsT=hT[:], rhs=wnext[k][:], start=True, stop=True)
                        tn = ep.tile([128, HID], BF16, tag="tb")
                        nc.scalar.activation(tn[:], ps2[:], AF.Copy, scale=dinv_t[:, b : b + 1])
                        nc.sync.dma_start(bnc[k + 1][b * 128 : (b + 1) * 128, :], tn[:])
                else:
                    for b in range(BLOCKS):
                        ohgl = ohp.tile([128, GB * 128], BF16, tag="ohg")
                        nc.sync.dma_start(ohgl[:], ohg_d[:, b * GB * 128 : (b + 1) * GB * 128])
                        for gb in range(GB):
                            gp = min(128, NG - gb * 128)
                            nc.tensor.matmul(
                                pool_ps[gb][:gp, :],
                                lhsT=ohgl[:, gb * 128 : gb * 128 + gp],
                                rhs=h3all[:, b * HID : (b + 1) * HID],
                                start=(b == 0),
                                stop=(b == BLOCKS - 1),
                            )
                nc.leave_named_scope(f"L{k}", sid, notify=False)
                if k < 2:
                    with nc.named_scope(f"ag{k+1}"):
                        nc.gpsimd.collective_compute(
                            "AllGather", OP.bypass, replica_groups=groups,
                            ins=[bnc[k + 1].ap().opt()], outs=[tab[k + 1].ap().opt()],
                        )

            # ---- pooling + final linear ----
            tid, _ = nc.enter_named_scope("tail", notify=False)
            for gb in range(GB):
                gp = min(128, NG - gb * 128)
                cpt = ep.tile([128, HID], F32, tag="t1")
                if gp < 128:
                    nc.any.memset(cpt[:], 0.0)
                nc.vector.tensor_copy(out=cpt[:gp, :], in_=pool_ps[gb][:gp, :])
                nc.sync.dma_start(pool_loc[gb * 128 : (gb + 1) * 128, :], cpt[:])
            nc.gpsimd.collective_compute(
                "AllReduce", OP.add, replica_groups=groups,
                ins=[pool_loc.ap().opt()], outs=[pool_sum.ap().opt()],
            )
            mx_t = ep.tile([128, GB], F32, tag="mx")
            nc.vector.tensor_scalar(mx_t[:], cnt_t[:], 1.0, None, OP.max)
            inv_t = ep.tile([128, GB], F32, tag="inv")
            nc.vector.reciprocal(inv_t[:], mx_t[:])
            for gb in range(GB):
                gp = min(128, NG - gb * 128)
                sm = ep.tile([128, HID], F32, tag="t1")
                nc.sync.dma_start(sm[:], pool_sum[gb * 128 : (gb + 1) * 128, :])
                mean = ep.tile([128, HID], F32, tag="h")
                nc.vector.tensor_scalar(mean[:], sm[:], inv_t[:, gb : gb + 1], None, OP.mult)
                pt = pst.tile([128, 128], F32, tag="pst")
                nc.tensor.transpose(pt[:HID, :], mean[:], ident[:])
                mT = ep.tile([HID, 128], F32, tag="hT")
                nc.scalar.copy(mT[:], pt[:HID, :])
                psf = psw.tile([128, NCLS], F32, tag="psw")
                nc.tensor.matmul(psf[:gp, :], lhsT=mT[:, :gp], rhs=wl_t[:], start=True, stop=True)
                of = ep.tile([128, NCLS], F32, tag="of")
                nc.vector.tensor_tensor(out=of[:gp, :], in0=psf[:gp, :], in1=blt_t[:gp, :], op=OP.add)
                nc.sync.dma_start(out_d[gb * 128 : gb * 128 + gp, :], of[:gp, :])
            nc.leave_named_scope("tail", tid, notify=False)

    nc.compile()
    return nc


def ix_max(t):
    return max(sum(t.Lsr[si][r] // 16 for r in range(R)) for si in range(len(t.sws)))


def dl_max(t):
    return max(int(t.chunks[sw, :].sum()) for sw in t.sws)


def m_max(t):
    return max(max(t.Lsr[si][r] // 128 for r in range(R)) for si in range(len(t.sws)))


def oh_max(t):
    return int(t.chunks.max())


# --------------------------------------------------------------------------
# Entry points
# --------------------------------------------------------------------------

def run_on_hw(inputs, cfg, trace=None):
    from concourse.bass_utils import run_bass_kernel_spmd

    if trace is None:
        trace = os.environ.get("GCN_TRACE", "0") == "1"
    t = preprocess(np.asarray(inputs["x"]), np.asarray(inputs["edge_index"]),
                   np.asarray(inputs["batch"]), cfg)
    in_maps = make_in_maps(
        t, *(np.asarray(inputs[k]) for k in
             ("W1", "b1", "W2", "b2", "W3", "b3", "Wlin", "blin")))
    nc = build_program(t)
    res = run_bass_kernel_spmd(nc, in_maps, core_ids=list(range(cfg["n_cores"])), trace=trace)
    run_on_hw.last = res
    return res.results[0]["out"].astype(np.float32)


def kernel(**inputs) -> np.ndarray:
    return run_on_hw(inputs, CFG_FULL)


# revision 30
# speedup vs baseline: 1.0575x; 1.0575x over previous
"""AdvancedGCN (3-layer GCNConv + global_mean_pool + linear) on 8 Trainium2
NeuronCores via Bass/Tile.

Strategy (per 8-way node sharding of dst nodes):
  - GCN layer out[d] = dinv[d] * sum_{e: dst=d} (h[src]@W)*dinv[src] + b
    with self-loops folded in as explicit (n->n) edges.
  - Per layer: each core owns a 12.5k-node shard of dst nodes. The projected,
    dinv-prescaled feature table (N x 64, f32) lives in HBM, AllGathered
    across cores each layer.
  - Edge messages are fetched with dma_gather (one 256B descriptor per edge,
    int16 indices; the 100k-row table is addressed via 4 "residue" views of
    stride 1024B so indices fit int16).
  - The segment-sum over dst is a PE matmul: per 128-edge chunk, a one-hot
    [128 edges x 128 dst] matrix (built on DVE via iota==dst_local compare)
    scatter-adds messages into a PSUM accumulator per 128-dst block.
  - Graph mean-pool is another one-hot matmul; partial sums are AllReduced.

Host-side work is limited to integer index bookkeeping: bucketing edges by
(dst-block, src-residue), padding to 128-slot chunks, degree/graph counts
(np.bincount). All float math runs on device.
"""
import os
import sys
import types

sys.path.insert(0, "/opt/trn_rl_repo")

import numpy as np
import ml_dtypes

BF = ml_dtypes.bfloat16


def _install_ntff_hook():
    """The agent image's antenv lacks axon_hooks; fabricate it so
    run_bass_kernel_spmd(trace=True) can capture NTFF profiles."""
    try:
        import antenv
    except ImportError:
        return
    if "antenv.axon_hooks" in sys.modules:
        return
    mod = types.ModuleType("antenv.axon_hooks")
    mod._hook = None

    def set_axon_ntff_profile_hook(h):
        mod._hook = h

    def get_axon_ntff_profile_hook():
        return mod._hook

    mod.set_axon_ntff_profile_hook = set_axon_ntff_profile_hook
    mod.get_axon_ntff_profile_hook = get_axon_ntff_profile_hook
    sys.modules["antenv.axon_hooks"] = mod
    antenv.axon_hooks = mod
    try:
        from trn_agent_boot.trn_boot import _ntff_profile_via_ctypes

        hook = _ntff_profile_via_ctypes("/opt/axon/libaxon_pjrt.so")
        if hook is not None:
            mod._hook = hook
    except Exception:
        pass


_install_ntff_hook()

import concourse.bass as bass
import concourse.bacc as bacc
import concourse.mybir as mybir
import concourse.tile as tile
from concourse._compat import cdiv
from concourse.library_config import mlp
from concourse.masks import make_identity

F32 = mybir.dt.float32
BF16 = mybir.dt.bfloat16
I16 = mybir.dt.int16
AF = mybir.ActivationFunctionType
OP = mybir.AluOpType

CFG_FULL = dict(
    n_nodes=100000,
    n_graphs=256,
    d_in=128,
    hid=64,
    n_cls=10,
    n_cores=8,
    sw=2,  # dst blocks per gather super-window
)

R = 4  # src residues (table addressed as [N/4, 4*hid] so idx fits int16)
SINGLE_PACKET = False  # >64 descriptors per lane overflows a single packet


# --------------------------------------------------------------------------
# Host preprocessing: integer bucketing / template construction
# --------------------------------------------------------------------------

class T:
    """Template: program-shape constants + per-core input arrays."""


def preprocess(x, edge_index, batch, cfg):
    t = T()
    NC = cfg["n_cores"]
    N = cfg["n_nodes"]
    HID = cfg["hid"]
    NG = cfg["n_graphs"]
    assert N % NC == 0
    NPC = N // NC
    BLOCKS = cdiv(NPC, 128)
    SH = BLOCKS * 128
    NTOT = NC * SH
    assert NTOT % R == 0 and NTOT // R <= 32768

    t.cfg = cfg
    t.NPC, t.BLOCKS, t.SH, t.NTOT = NPC, BLOCKS, SH, NTOT
    t.GB = cdiv(NG, 128)

    src = edge_index[0].astype(np.int64)
    dst = edge_index[1].astype(np.int64)
    deg = (np.bincount(dst, minlength=N) + 1).astype(np.float32)

    S = src
    D = dst
    gsrc = (S // NPC) * SH + (S % NPC)  # padded global table row of src
    core = D // NPC
    dloc = D % NPC
    blk = dloc // 128
    dl = (dloc % 128).astype(np.float32)
    res = gsrc % R
    qidx = (gsrc // R).astype(np.int16)

    ncells = NC * BLOCKS * R
    key = ((core * BLOCKS + blk) * R + res).astype(np.int64)
    order = np.argsort(key, kind="stable")
    q_sorted = qidx[order]
    dl_sorted = dl[order]
    counts = np.bincount(key, minlength=ncells).reshape(NC, BLOCKS, R)
    starts = np.zeros(ncells + 1, dtype=np.int64)
    np.cumsum(counts.reshape(-1), out=starts[1:])

    chunks = np.ceil(counts.max(axis=0) / 128).astype(np.int64)  # [BLOCKS, R]
    slots = chunks * 128
    t.chunks = chunks

    # super-windows
    SW = cfg["sw"]
    t.sws = [list(range(i, min(i + SW, BLOCKS))) for i in range(0, BLOCKS, SW)]

    # gather-call sizes and idx column offsets (order: si asc, r asc)
    t.Lsr = [[int(slots[sw, r].sum()) for r in range(R)] for sw in t.sws]
    icol = []
    off = 0
    for si in range(len(t.sws)):
        icol.append([])
        for r in range(R):
            icol[si].append(off)
            off += t.Lsr[si][r] // 16
    t.icol, t.TOTC = icol, off

    # dlocal instance column offsets (order: b asc, r asc, j asc)
    dcol = np.zeros((BLOCKS, R), dtype=np.int64)
    off = 0
    for b in range(BLOCKS):
        for r in range(R):
            dcol[b, r] = off
            off += chunks[b, r]
    t.dcol, t.TOTI = dcol, int(off)

    # msgs column base of (b, r) within its super-window's residue-r buffer
    mcol = np.zeros((BLOCKS, R), dtype=np.int64)
    for sw in t.sws:
        for r in range(R):
            off = 0
            for b in sw:
                mcol[b, r] = off
                off += chunks[b, r]
    t.mcol = mcol

    # ---- per-core arrays ----
    xT = x.T.astype(np.float32)  # [d_in, N]
    t.per_core = []
    for c in range(NC):
        qpad = {}
        dlp = {}
        for b in range(BLOCKS):
            for r in range(R):
                m = (c * BLOCKS + b) * R + r
                s, e = starts[m], starts[m + 1]
                L = int(slots[b, r])
                qq = np.zeros(L, dtype=np.int16)
                dd = np.full(L, -1.0, dtype=np.float32)
                qq[: e - s] = q_sorted[s:e]
                dd[: e - s] = dl_sorted[s:e]
                qpad[(b, r)] = qq
                dlp[(b, r)] = dd

        gidx = np.zeros((128, t.TOTC), dtype=np.int16)
        for si, sw in enumerate(t.sws):
            for r in range(R):
                v = np.concatenate([qpad[(b, r)] for b in sw])
                w = v.reshape(-1, 16).T  # [16, L/16]; pos i -> (i%16, i//16)
                gidx[:, icol[si][r] : icol[si][r] + v.size // 16] = np.tile(w, (8, 1))

        # host-built one-hot scatter matrices: [TOTI, 128 slot, 128 dst] bf16,
        # laid out as [128 slot-partitions, TOTI*128] for DMA into SBUF.
        ohm = np.zeros((t.TOTI, 128, 128), dtype=BF)
        for b in range(BLOCKS):
            for r in range(R):
                dd = dlp[(b, r)].astype(np.int64)  # [slots]; -1 = pad
                for j in range(int(chunks[b, r])):
                    seg = dd[j * 128 : (j + 1) * 128]
                    val = seg >= 0
                    ohm[dcol[b, r] + j][np.nonzero(val)[0], seg[val]] = 1
        ohm = np.ascontiguousarray(ohm.swapaxes(0, 1).reshape(128, t.TOTI * 128))

        lo, hi = c * NPC, (c + 1) * NPC
        xTs = np.zeros((cfg["d_in"], SH), dtype=np.float32)
        xTs[:, :NPC] = xT[:, lo:hi]

        degs = np.ones(SH, dtype=np.float32)
        degs[:NPC] = deg[lo:hi]
        degw = degs.reshape(BLOCKS, 128).T.copy()  # [128, BLOCKS]

        bats = np.full(SH, -1, dtype=np.int64)
        bats[:NPC] = batch[lo:hi]
        batw = bats.reshape(BLOCKS, 128).T  # [128, BLOCKS]
        # host pool one-hots: [128, (b*GB+gb)*128 + c] = (batch == gb*128+c)
        ohg = np.zeros((128, BLOCKS * t.GB * 128), dtype=BF)
        for b in range(BLOCKS):
            for gb in range(t.GB):
                eq = batw[:, b : b + 1] == (gb * 128 + np.arange(128))[None, :]
                ohg[:, (b * t.GB + gb) * 128 : (b * t.GB + gb + 1) * 128] = eq
        t.per_core.append(dict(gidx=gidx, oh=ohm, ohg=ohg, xT=xTs, deg=degw))

    cnt = np.bincount(batch.astype(np.int64), minlength=NG).astype(np.float32)
    cntw = np.zeros((128, t.GB), dtype=np.float32)
    for gb in range(t.GB):
        n = min(128, NG - gb * 128)
        cntw[:n, gb] = cnt[gb * 128 : gb * 128 + n]
    t.cnt = cntw

    return t


def make_in_maps(t, W1, b1, W2, b2, W3, b3, Wlin, blin):
    cfg = t.cfg
    HID = cfg["hid"]
    shared = dict(
        cnt=t.cnt,
        w1=W1.astype(np.float32),
        w2=W2.astype(np.float32),
        w3=W3.astype(np.float32),
        wl=Wlin.astype(np.float32),
        b1t=np.tile(b1.astype(np.float32), (128, 1)),
        b2t=np.tile(b2.astype(np.float32), (128, 1)),
        b3t=np.tile(b3.astype(np.float32), (128, 1)),
        blt=np.tile(blin.astype(np.float32), (128, 1)),
    )
    return [dict(shared, **pc) for pc in t.per_core]


# --------------------------------------------------------------------------
# Device program
# --------------------------------------------------------------------------

def build_program(t, enable_asserts=False):
    cfg = t.cfg
    NC = cfg["n_cores"]
    HID = cfg["hid"]
    DIN = cfg["d_in"]
    NG = cfg["n_graphs"]
    NCLS = cfg["n_cls"]
    BLOCKS, SH, NTOT, GB = t.BLOCKS, t.SH, t.NTOT, t.GB
    chunks, Lsr, icol, dcol, mcol = t.chunks, t.Lsr, t.icol, t.dcol, t.mcol
    IW = max(256, 128 * GB)

    nc = bacc.Bacc(
        "TRN2",
        target_bir_lowering=False,
        debug=False,
        enable_asserts=enable_asserts,
        num_devices=NC,
        num_swdge_queues=4,
    )

    din = lambda n, s, d=F32: nc.dram_tensor(n, s, d, kind="ExternalInput")
    xT_d = din("xT", [DIN, SH])
    gidx_d = din("gidx", [128, t.TOTC], I16)
    ohd = din("oh", [128, t.TOTI * 128], BF16)
    deg_d = din("deg", [128, BLOCKS])
    ohg_d = din("ohg", [128, BLOCKS * GB * 128], BF16)
    cnt_d = din("cnt", [128, GB])
    w1_d = din("w1", [DIN, HID])
    w2_d = din("w2", [HID, HID])
    w3_d = din("w3", [HID, HID])
    wl_d = din("wl", [HID, NCLS])
    b1t_d = din("b1t", [128, HID])
    b2t_d = din("b2t", [128, HID])
    b3t_d = din("b3t", [128, HID])
    blt_d = din("blt", [128, NCLS])
    out_d = nc.dram_tensor("out", [NG, NCLS], F32, kind="ExternalOutput")

    tab = [nc.dram_tensor(f"table{k}", [NTOT, HID], BF16, addr_space="Shared") for k in range(3)]
    bnc = [nc.dram_tensor(f"bounce{k}", [SH, HID], BF16) for k in range(3)]
    pool_loc = nc.dram_tensor("pool_loc", [128 * GB, HID], F32)
    pool_sum = nc.dram_tensor("pool_sum", [128 * GB, HID], F32, addr_space="Shared")

    groups = [list(range(NC))]

    with tile.TileContext(nc) as tc:
        with (
            tc.tile_pool(name="const", bufs=1) as cp,
            tc.tile_pool(name="xw", bufs=3) as xp,
            tc.tile_pool(name="ix", bufs=2) as ixp,
            tc.tile_pool(name="msg", bufs=2) as mp,
            tc.tile_pool(name="oh", bufs=2) as ohp,
            tc.tile_pool(name="hall", bufs=1) as hap,
            tc.tile_pool(name="ep", bufs=3) as ep,
            tc.tile_pool(name="psb", bufs=2, space="PSUM") as psb,
            tc.tile_pool(name="pst", bufs=2, space="PSUM") as pst,
            tc.tile_pool(name="psw", bufs=2, space="PSUM") as psw,
            tc.tile_pool(name="psg", bufs=1, space="PSUM") as psg,
        ):
            nc.gpsimd.load_library(mlp)

            # ---- constants ----
            cnt_t = cp.tile([128, GB], F32, tag="cnt")
            nc.sync.dma_start(cnt_t[:], cnt_d[:, :])
            deg_t = cp.tile([128, BLOCKS], F32, tag="deg")
            nc.sync.dma_start(deg_t[:], deg_d[:, :])
            dsq_t = cp.tile([128, BLOCKS], F32, tag="dsq")
            nc.scalar.activation(dsq_t[:], deg_t[:], AF.Sqrt)
            dinv_t = cp.tile([128, BLOCKS], F32, tag="dinv")
            nc.vector.reciprocal(dinv_t[:], dsq_t[:])
            w1_t = cp.tile([DIN, HID], F32, tag="w1")
            nc.sync.dma_start(w1_t[:], w1_d[:, :])
            w2_t = cp.tile([HID, HID], F32, tag="w2")
            nc.sync.dma_start(w2_t[:], w2_d[:, :])
            w3_t = cp.tile([HID, HID], F32, tag="w3")
            nc.sync.dma_start(w3_t[:], w3_d[:, :])
            wl_t = cp.tile([HID, NCLS], F32, tag="wl")
            nc.sync.dma_start(wl_t[:], wl_d[:, :])
            bt = []
            for nm, d in (("b1t", b1t_d), ("b2t", b2t_d), ("b3t", b3t_d)):
                b_ = cp.tile([128, HID], F32, tag=nm)
                nc.sync.dma_start(b_[:], d[:, :])
                bt.append(b_)
            blt_t = cp.tile([128, NCLS], F32, tag="blt")
            nc.sync.dma_start(blt_t[:], blt_d[:, :])
            ident = cp.tile([128, 128], F32, tag="ident")
            make_identity(nc, ident[:])

            wnext = [w2_t, w3_t]

            # ---- phase 1: table0 = (x @ W1) * dinv ----
            with nc.named_scope("p1"):
                for b in range(BLOCKS):
                    xt = xp.tile([DIN, 128], F32, tag="xt")
                    nc.sync.dma_start(xt[:], xT_d[:, b * 128 : (b + 1) * 128])
                    ps = psw.tile([128, HID], F32, tag="psw")
                    nc.tensor.matmul(ps[:], lhsT=xt[:], rhs=w1_t[:], start=True, stop=True)
                    tb = ep.tile([128, HID], BF16, tag="tb")
                    nc.scalar.activation(tb[:], ps[:], AF.Copy, scale=dinv_t[:, b : b + 1])
                    nc.sync.dma_start(bnc[0][b * 128 : (b + 1) * 128, :], tb[:])
            with nc.named_scope("ag0"):
                nc.gpsimd.collective_compute(
                    "AllGather", OP.bypass, replica_groups=groups,
                    ins=[bnc[0].ap().opt()], outs=[tab[0].ap().opt()],
                )

            # ---- layers ----
            pool_ps = None
            for k in range(3):
                # pair views of the bf16 gather table: each 256B descriptor
                # fetches 2 adjacent rows; class r = 2*rp + h, rp picks the
                # pair within a row-quad, h the row within the pair.
                tview = tab[k].ap().rearrange("(a b) d -> a (b d)", b=R)
                rviews = [tview[:, (r // 2) * 2 * HID : (r // 2 + 1) * 2 * HID] for r in range(R)]
                if k == 2:
                    pool_ps = []
                    for gb in range(GB):
                        pps = psg.tile([128, HID], F32, tag=f"psg{gb}")
                        pool_ps.append(pps)

                sid, _ = nc.enter_named_scope(f"L{k}", notify=False)
                if k < 2:
                    hall = hap.tile([128, BLOCKS * HID], F32, tag="hall")
                else:
                    h3all = hap.tile([128, BLOCKS * HID], BF16, tag="h3all")
                for si, sw in enumerate(t.sws):
                    icw = sum(Lsr[si][r] // 16 for r in range(R))
                    ixt = ixp.tile([128, max(ix_max(t), 16)], I16, tag="ixt")
                    nc.sync.dma_start(ixt[:, :icw], gidx_d[:, icol[si][0] : icol[si][0] + icw])
                    dw = int(chunks[sw, :].sum())
                    ohl = ohp.tile([128, max(dl_max(t), 1) * 128], BF16, tag="ohl")
                    d0 = int(dcol[sw[0], 0])
                    nc.sync.dma_start(ohl[:, : dw * 128], ohd[:, d0 * 128 : (d0 + dw) * 128])

                    gts = [None] * R
                    for r in (1, 2, 3, 0):  # queue 0 runs sync on Pool: dispatch last
                        L = Lsr[si][r]
                        cols = L // 128
                        gt = mp.tile([128, max(m_max(t), 1), 2 * HID], BF16, tag=f"m{r}", name=f"gt{r}")
                        if L:
                            a0 = icol[si][r] - icol[si][0]
                            nc.gpsimd.dma_gather(
                                gt[:, :cols, :], rviews[r], ixt[:, a0 : a0 + L // 16],
                                L, L, 2 * HID, elem_step=R * HID,
                                single_packet=SINGLE_PACKET, queue_num=r,
                            )
                        gts[r] = gt

                    for b in sw:
                        ps = psb.tile([128, HID], F32, tag="psb")
                        nch = int(chunks[b, :].sum())
                        done = 0
                        for r in range(R):
                            ch = int(chunks[b, r])
                            if ch == 0:
                                continue
                            c0 = int(dcol[b, r]) - int(dcol[sw[0], 0])
                            hoff = (r % 2) * HID
                            for j in range(ch):
                                nc.tensor.matmul(
                                    ps[:],
                                    lhsT=ohl[:, (c0 + j) * 128 : (c0 + j + 1) * 128],
                                    rhs=gts[r][:, mcol[b, r] + j, hoff : hoff + HID],
                                    start=(done == 0),
                                    stop=(done == nch - 1),
                                )
                                done += 1

                        # ---- block epilogue: h = dinv*ps + (dinv*town + bias) ----
                        townb = ep.tile([128, HID], BF16, tag="townb")
                        nc.sync.dma_start(townb[:], bnc[k][b * 128 : (b + 1) * 128, :])
                        pre = ep.tile([128, HID], F32, tag="pre")
                        nc.scalar.activation(pre[:], townb[:], AF.Copy, scale=dinv_t[:, b : b + 1])
                        pre2 = ep.tile([128, HID], F32, tag="pre2")
                        nc.vector.tensor_add(pre2[:], pre[:], bt[k][:])
                        t1 = ep.tile([128, HID], F32, tag="t1")
                        nc.scalar.activation(t1[:], ps[:], AF.Copy, scale=dinv_t[:, b : b + 1])
                        hsl = slice(b * HID, (b + 1) * HID)
                        hp = ep.tile([128, HID], F32, tag="hp")
                        nc.vector.tensor_add(hp[:], t1[:], pre2[:])
                        if k < 2:
                            nc.scalar.activation(hall[:, hsl], hp[:], AF.Relu)
                        else:
                            nc.scalar.copy(h3all[:, hsl], hp[:])

                # ---- deferred PE phase: next-layer projection / pooling ----
                if k < 2:
                    for b in range(BLOCKS):
                        pt = pst.tile([128, 128], F32, tag="pst")
                        nc.tensor.transpose(pt[:HID, :], hall[:, b * HID : (b + 1) * HID], ident[:])
                        hT = ep.tile([HID, 128], F32, tag="hT")
                        nc.scalar.copy(hT[:], pt[:HID, :])
                        ps2 = psw.tile([128, HID], F32, tag="psw")
                        nc.tensor.matmul(ps2[:], lhsT=hT[:], rhs=wnext[k][:], start=True, stop=True)
                        tn = ep.tile([128, HID], BF16, tag="tb")
                        nc.scalar.activation(tn[:], ps2[:], AF.Copy, scale=dinv_t[:, b : b + 1])
                        nc.sync.dma_start(bnc[k + 1][b * 128 : (b + 1) * 128, :], tn[:])
                else:
                    for b in range(BLOCKS):
                        ohgl = ohp.tile([128, GB * 128], BF16, tag="ohg")
                        nc.sync.dma_start(ohgl[:], ohg_d[:, b * GB * 128 : (b + 1) * GB * 128])
                        for gb in range(GB):
                            gp = min(128, NG - gb * 128)
                            nc.tensor.matmul(
                                pool_ps[gb][:gp, :],
                                lhsT=ohgl[:, gb * 128 : gb * 128 + gp],
                                rhs=h3all[:, b * HID : (b + 1) * HID],
                                start=(b == 0),
                                stop=(b == BLOCKS - 1),
                            )
                nc.leave_named_scope(f"L{k}", sid, notify=False)
                if k < 2:
                    with nc.named_scope(f"ag{k+1}"):
                        nc.gpsimd.collective_compute(
                            "AllGather", OP.bypass, replica_groups=groups,
                            ins=[bnc[k + 1].ap().opt()], outs=[tab[k + 1].ap().opt()],
                        )

            # ---- pooling + final linear ----
            tid, _ = nc.enter_named_scope("tail", notify=False)
            for gb in range(GB):
                gp = min(128, NG - gb * 128)
                cpt = ep.tile([128, HID], F32, tag="t1")
                if gp < 128:
                    nc.any.memset(cpt[:], 0.0)
                nc.vector.tensor_copy(out=cpt[:gp, :], in_=pool_ps[gb][:gp, :])
                nc.sync.dma_start(pool_loc[gb * 128 : (gb + 1) * 128, :], cpt[:])
            nc.gpsimd.collective_compute(
                "AllReduce", OP.add, replica_groups=groups,
                ins=[pool_loc.ap().opt()], outs=[pool_sum.ap().opt()],
            )
            mx_t = ep.tile([128, GB], F32, tag="mx")
            nc.vector.tensor_scalar(mx_t[:], cnt_t[:], 1.0, None, OP.max)
            inv_t = ep.tile([128, GB], F32, tag="inv")
            nc.vector.reciprocal(inv_t[:], mx_t[:])
            for gb in range(GB):
                gp = min(128, NG - gb * 128)
                sm = ep.tile([128, HID], F32, tag="t1")
                nc.sync.dma_start(sm[:], pool_sum[gb * 128 : (gb + 1) * 128, :])
                mean = ep.tile([128, HID], F32, tag="h")
                nc.vector.tensor_scalar(mean[:], sm[:], inv_t[:, gb : gb + 1], None, OP.mult)
                pt = pst.tile([128, 128], F32, tag="pst")
                nc.tensor.transpose(pt[:HID, :], mean[:], ident[:])
                mT = ep.tile([HID, 128], F32, tag="hT")
                nc.scalar.copy(mT[:], pt[:HID, :])
                psf = psw.tile([128, NCLS], F32, tag="psw")
                nc.tensor.matmul(psf[:gp, :], lhsT=mT[:, :gp], rhs=wl_t[:], start=True, stop=True)
                of = ep.tile([128, NCLS], F32, tag="of")
                nc.vector.tensor_tensor(out=of[:gp, :], in0=psf[:gp, :], in1=blt_t[:gp, :], op=OP.add)
                nc.sync.dma_start(out_d[gb * 128 : gb * 128 + gp, :], of[:gp, :])
            nc.leave_named_scope("tail", tid, notify=False)

    nc.compile()
    return nc


def ix_max(t):
    return max(sum(t.Lsr[si][r] // 16 for r in range(R)) for si in range(len(t.sws)))


def dl_max(t):
    return max(int(t.chunks[sw, :].sum()) for sw in t.sws)


def m_max(t):
    return max(max(t.Lsr[si][r] // 128 for r in range(R)) for si in range(len(t.sws)))


def oh_max(t):
    return int(t.chunks.max())


# --------------------------------------------------------------------------
# Entry points
# --------------------------------------------------------------------------

def run_on_hw(inputs, cfg, trace=None):
    from concourse.bass_utils import run_bass_kernel_spmd

    if trace is None:
        trace = os.environ.get("GCN_TRACE", "0") == "1"
    t = preprocess(np.asarray(inputs["x"]), np.asarray(inputs["edge_index"]),
                   np.asarray(inputs["batch"]), cfg)
    in_maps = make_in_maps(
        t, *(np.asarray(inputs[k]) for k in
             ("W1", "b1", "W2", "b2", "W3", "b3", "Wlin", "blin")))
    nc = build_program(t)
    res = run_bass_kernel_spmd(nc, in_maps, core_ids=list(range(cfg["n_cores"])), trace=trace)
    run_on_hw.last = res
    return res.results[0]["out"].astype(np.float32)


def kernel(**inputs) -> np.ndarray:
    return run_on_hw(inputs, CFG_FULL)


# revision 31
# speedup vs baseline: 1.1434x; 1.0812x over previous
"""AdvancedGCN (3-layer GCNConv + global_mean_pool + linear) on 8 Trainium2
NeuronCores via Bass/Tile.

Strategy (per 8-way node sharding of dst nodes):
  - GCN layer out[d] = dinv[d] * sum_{e: dst=d} (h[src]@W)*dinv[src] + b
    with self-loops folded in as explicit (n->n) edges.
  - Per layer: each core owns a 12.5k-node shard of dst nodes. The projected,
    dinv-prescaled feature table (N x 64, f32) lives in HBM, AllGathered
    across cores each layer.
  - Edge messages are fetched with dma_gather (one 256B descriptor per edge,
    int16 indices; the 100k-row table is addressed via 4 "residue" views of
    stride 1024B so indices fit int16).
  - The segment-sum over dst is a PE matmul: per 128-edge chunk, a one-hot
    [128 edges x 128 dst] matrix (built on DVE via iota==dst_local compare)
    scatter-adds messages into a PSUM accumulator per 128-dst block.
  - Graph mean-pool is another one-hot matmul; partial sums are AllReduced.

Host-side work is limited to integer index bookkeeping: bucketing edges by
(dst-block, src-residue), padding to 128-slot chunks, degree/graph counts
(np.bincount). All float math runs on device.
"""
import os
import sys
import types

sys.path.insert(0, "/opt/trn_rl_repo")

import numpy as np
import ml_dtypes

BF = ml_dtypes.bfloat16


def _install_ntff_hook():
    """The agent image's antenv lacks axon_hooks; fabricate it so
    run_bass_kernel_spmd(trace=True) can capture NTFF profiles."""
    try:
        import antenv
    except ImportError:
        return
    if "antenv.axon_hooks" in sys.modules:
        return
    mod = types.ModuleType("antenv.axon_hooks")
    mod._hook = None

    def set_axon_ntff_profile_hook(h):
        mod._hook = h

    def get_axon_ntff_profile_hook():
        return mod._hook

    mod.set_axon_ntff_profile_hook = set_axon_ntff_profile_hook
    mod.get_axon_ntff_profile_hook = get_axon_ntff_profile_hook
    sys.modules["antenv.axon_hooks"] = mod
    antenv.axon_hooks = mod
    try:
        from trn_agent_boot.trn_boot import _ntff_profile_via_ctypes

        hook = _ntff_profile_via_ctypes("/opt/axon/libaxon_pjrt.so")
        if hook is not None:
            mod._hook = hook
    except Exception:
        pass


_install_ntff_hook()

import concourse.bass as bass
import concourse.bacc as bacc
import concourse.mybir as mybir
import concourse.tile as tile
from concourse._compat import cdiv
from concourse.library_config import mlp
from concourse.masks import make_identity

F32 = mybir.dt.float32
BF16 = mybir.dt.bfloat16
I16 = mybir.dt.int16
AF = mybir.ActivationFunctionType
OP = mybir.AluOpType

CFG_FULL = dict(
    n_nodes=100000,
    n_graphs=256,
    d_in=128,
    hid=64,
    n_cls=10,
    n_cores=8,
    sw=2,  # dst blocks per gather super-window
)

R = 4  # src residues (table addressed as [N/4, 4*hid] so idx fits int16)
SINGLE_PACKET = False  # >64 descriptors per lane overflows a single packet


# --------------------------------------------------------------------------
# Host preprocessing: integer bucketing / template construction
# --------------------------------------------------------------------------

class T:
    """Template: program-shape constants + per-core input arrays."""


def preprocess(x, edge_index, batch, cfg):
    t = T()
    NC = cfg["n_cores"]
    N = cfg["n_nodes"]
    HID = cfg["hid"]
    NG = cfg["n_graphs"]
    assert N % NC == 0
    NPC = N // NC
    BLOCKS = cdiv(NPC, 128)
    SH = BLOCKS * 128
    NTOT = NC * SH
    assert NTOT % R == 0 and NTOT // R <= 32768

    t.cfg = cfg
    t.NPC, t.BLOCKS, t.SH, t.NTOT = NPC, BLOCKS, SH, NTOT
    t.GB = cdiv(NG, 128)

    src = edge_index[0].astype(np.int64)
    dst = edge_index[1].astype(np.int64)
    deg = (np.bincount(dst, minlength=N) + 1).astype(np.float32)

    S = src
    D = dst
    gsrc = (S // NPC) * SH + (S % NPC)  # padded global table row of src
    core = D // NPC
    dloc = D % NPC
    blk = dloc // 128
    dl = (dloc % 128).astype(np.float32)
    res = gsrc % R
    qidx = (gsrc // R).astype(np.int16)

    ncells = NC * BLOCKS * R
    key = ((core * BLOCKS + blk) * R + res).astype(np.int64)
    order = np.argsort(key, kind="stable")
    q_sorted = qidx[order]
    dl_sorted = dl[order]
    counts = np.bincount(key, minlength=ncells).reshape(NC, BLOCKS, R)
    starts = np.zeros(ncells + 1, dtype=np.int64)
    np.cumsum(counts.reshape(-1), out=starts[1:])

    chunks = np.ceil(counts.max(axis=0) / 128).astype(np.int64)  # [BLOCKS, R]
    slots = chunks * 128
    t.chunks = chunks

    # super-windows
    SW = cfg["sw"]
    t.sws = [list(range(i, min(i + SW, BLOCKS))) for i in range(0, BLOCKS, SW)]

    # gather-call sizes and idx column offsets (order: si asc, r asc)
    t.Lsr = [[int(slots[sw, r].sum()) for r in range(R)] for sw in t.sws]
    icol = []
    off = 0
    for si in range(len(t.sws)):
        icol.append([])
        for r in range(R):
            icol[si].append(off)
            off += t.Lsr[si][r] // 16
    t.icol, t.TOTC = icol, off

    # dlocal instance column offsets (order: b asc, r asc, j asc)
    dcol = np.zeros((BLOCKS, R), dtype=np.int64)
    off = 0
    for b in range(BLOCKS):
        for r in range(R):
            dcol[b, r] = off
            off += chunks[b, r]
    t.dcol, t.TOTI = dcol, int(off)

    # msgs column base of (b, r) within its super-window's residue-r buffer
    mcol = np.zeros((BLOCKS, R), dtype=np.int64)
    for sw in t.sws:
        for r in range(R):
            off = 0
            for b in sw:
                mcol[b, r] = off
                off += chunks[b, r]
    t.mcol = mcol

    # ---- per-core arrays ----
    xT = x.T.astype(np.float32)  # [d_in, N]
    t.per_core = []
    for c in range(NC):
        qpad = {}
        dlp = {}
        for b in range(BLOCKS):
            for r in range(R):
                m = (c * BLOCKS + b) * R + r
                s, e = starts[m], starts[m + 1]
                L = int(slots[b, r])
                qq = np.zeros(L, dtype=np.int16)
                dd = np.full(L, -1.0, dtype=np.float32)
                qq[: e - s] = q_sorted[s:e]
                dd[: e - s] = dl_sorted[s:e]
                qpad[(b, r)] = qq
                dlp[(b, r)] = dd

        gidx = np.zeros((128, t.TOTC), dtype=np.int16)
        for si, sw in enumerate(t.sws):
            for r in range(R):
                v = np.concatenate([qpad[(b, r)] for b in sw])
                w = v.reshape(-1, 16).T  # [16, L/16]; pos i -> (i%16, i//16)
                gidx[:, icol[si][r] : icol[si][r] + v.size // 16] = np.tile(w, (8, 1))

        # host-built one-hot scatter matrices: [TOTI, 128 slot, 128 dst] bf16,
        # laid out as [128 slot-partitions, TOTI*128] for DMA into SBUF.
        ohm = np.zeros((t.TOTI, 128, 128), dtype=BF)
        for b in range(BLOCKS):
            for r in range(R):
                dd = dlp[(b, r)].astype(np.int64)  # [slots]; -1 = pad
                for j in range(int(chunks[b, r])):
                    seg = dd[j * 128 : (j + 1) * 128]
                    val = seg >= 0
                    ohm[dcol[b, r] + j][np.nonzero(val)[0], seg[val]] = 1
        ohm = np.ascontiguousarray(ohm.swapaxes(0, 1).reshape(128, t.TOTI * 128))

        lo, hi = c * NPC, (c + 1) * NPC
        xTs = np.zeros((cfg["d_in"], SH), dtype=np.float32)
        xTs[:, :NPC] = xT[:, lo:hi]

        degs = np.ones(SH, dtype=np.float32)
        degs[:NPC] = deg[lo:hi]
        degw = degs.reshape(BLOCKS, 128).T.copy()  # [128, BLOCKS]

        bats = np.full(SH, -1, dtype=np.int64)
        bats[:NPC] = batch[lo:hi]
        batw = bats.reshape(BLOCKS, 128).T  # [128, BLOCKS]
        # host pool one-hots: [128, (b*GB+gb)*128 + c] = (batch == gb*128+c)
        ohg = np.zeros((128, BLOCKS * t.GB * 128), dtype=BF)
        for b in range(BLOCKS):
            for gb in range(t.GB):
                eq = batw[:, b : b + 1] == (gb * 128 + np.arange(128))[None, :]
                ohg[:, (b * t.GB + gb) * 128 : (b * t.GB + gb + 1) * 128] = eq
        t.per_core.append(dict(gidx=gidx, oh=ohm, ohg=ohg, xT=xTs, deg=degw))

    cnt = np.bincount(batch.astype(np.int64), minlength=NG).astype(np.float32)
    cntw = np.zeros((128, t.GB), dtype=np.float32)
    for gb in range(t.GB):
        n = min(128, NG - gb * 128)
        cntw[:n, gb] = cnt[gb * 128 : gb * 128 + n]
    t.cnt = cntw

    return t


def make_in_maps(t, W1, b1, W2, b2, W3, b3, Wlin, blin):
    cfg = t.cfg
    HID = cfg["hid"]
    shared = dict(
        cnt=t.cnt,
        w1=W1.astype(np.float32),
        w2=W2.astype(np.float32),
        w3=W3.astype(np.float32),
        wl=Wlin.astype(np.float32),
        b1t=np.tile(b1.astype(np.float32), (128, 1)),
        b2t=np.tile(b2.astype(np.float32), (128, 1)),
        b3t=np.tile(b3.astype(np.float32), (128, 1)),
        blt=np.tile(blin.astype(np.float32), (128, 1)),
    )
    return [dict(shared, **pc) for pc in t.per_core]


# --------------------------------------------------------------------------
# Device program
# --------------------------------------------------------------------------

def build_program(t, enable_asserts=False):
    cfg = t.cfg
    NC = cfg["n_cores"]
    HID = cfg["hid"]
    DIN = cfg["d_in"]
    NG = cfg["n_graphs"]
    NCLS = cfg["n_cls"]
    BLOCKS, SH, NTOT, GB = t.BLOCKS, t.SH, t.NTOT, t.GB
    chunks, Lsr, icol, dcol, mcol = t.chunks, t.Lsr, t.icol, t.dcol, t.mcol
    IW = max(256, 128 * GB)

    nc = bacc.Bacc(
        "TRN2",
        target_bir_lowering=False,
        debug=False,
        enable_asserts=enable_asserts,
        num_devices=NC,
        num_swdge_queues=4,
    )

    din = lambda n, s, d=F32: nc.dram_tensor(n, s, d, kind="ExternalInput")
    xT_d = din("xT", [DIN, SH])
    gidx_d = din("gidx", [128, t.TOTC], I16)
    ohd = din("oh", [128, t.TOTI * 128], BF16)
    deg_d = din("deg", [128, BLOCKS])
    ohg_d = din("ohg", [128, BLOCKS * GB * 128], BF16)
    cnt_d = din("cnt", [128, GB])
    w1_d = din("w1", [DIN, HID])
    w2_d = din("w2", [HID, HID])
    w3_d = din("w3", [HID, HID])
    wl_d = din("wl", [HID, NCLS])
    b1t_d = din("b1t", [128, HID])
    b2t_d = din("b2t", [128, HID])
    b3t_d = din("b3t", [128, HID])
    blt_d = din("blt", [128, NCLS])
    out_d = nc.dram_tensor("out", [NG, NCLS], F32, kind="ExternalOutput")

    tab = [nc.dram_tensor(f"table{k}", [NTOT, HID], BF16, addr_space="Shared") for k in range(3)]
    bnc = [nc.dram_tensor(f"bounce{k}", [SH, HID], BF16) for k in range(3)]
    pool_loc = nc.dram_tensor("pool_loc", [128 * GB, HID], F32)
    pool_sum = nc.dram_tensor("pool_sum", [128 * GB, HID], F32, addr_space="Shared")

    groups = [list(range(NC))]

    with tile.TileContext(nc) as tc:
        with (
            tc.tile_pool(name="const", bufs=1) as cp,
            tc.tile_pool(name="xw", bufs=3) as xp,
            tc.tile_pool(name="ix", bufs=3) as ixp,
            tc.tile_pool(name="msg", bufs=3) as mp,
            tc.tile_pool(name="oh", bufs=3) as ohp,
            tc.tile_pool(name="hall", bufs=1) as hap,
            tc.tile_pool(name="ep", bufs=3) as ep,
            tc.tile_pool(name="psb", bufs=2, space="PSUM") as psb,
            tc.tile_pool(name="pst", bufs=2, space="PSUM") as pst,
            tc.tile_pool(name="psw", bufs=2, space="PSUM") as psw,
            tc.tile_pool(name="psg", bufs=1, space="PSUM") as psg,
        ):
            nc.gpsimd.load_library(mlp)

            # ---- constants ----
            cnt_t = cp.tile([128, GB], F32, tag="cnt")
            nc.sync.dma_start(cnt_t[:], cnt_d[:, :])
            deg_t = cp.tile([128, BLOCKS], F32, tag="deg")
            nc.sync.dma_start(deg_t[:], deg_d[:, :])
            dsq_t = cp.tile([128, BLOCKS], F32, tag="dsq")
            nc.scalar.activation(dsq_t[:], deg_t[:], AF.Sqrt)
            dinv_t = cp.tile([128, BLOCKS], F32, tag="dinv")
            nc.vector.reciprocal(dinv_t[:], dsq_t[:])
            w1_t = cp.tile([DIN, HID], F32, tag="w1")
            nc.sync.dma_start(w1_t[:], w1_d[:, :])
            w2_t = cp.tile([HID, HID], F32, tag="w2")
            nc.sync.dma_start(w2_t[:], w2_d[:, :])
            w3_t = cp.tile([HID, HID], F32, tag="w3")
            nc.sync.dma_start(w3_t[:], w3_d[:, :])
            wl_t = cp.tile([HID, NCLS], F32, tag="wl")
            nc.sync.dma_start(wl_t[:], wl_d[:, :])
            bt = []
            for nm, d in (("b1t", b1t_d), ("b2t", b2t_d), ("b3t", b3t_d)):
                b_ = cp.tile([128, HID], F32, tag=nm)
                nc.sync.dma_start(b_[:], d[:, :])
                bt.append(b_)
            blt_t = cp.tile([128, NCLS], F32, tag="blt")
            nc.sync.dma_start(blt_t[:], blt_d[:, :])
            ident = cp.tile([128, 128], F32, tag="ident")
            make_identity(nc, ident[:])

            wnext = [w2_t, w3_t]

            # ---- phase 1: table0 = (x @ W1) * dinv ----
            with nc.named_scope("p1"):
                for b in range(BLOCKS):
                    xt = xp.tile([DIN, 128], F32, tag="xt")
                    nc.sync.dma_start(xt[:], xT_d[:, b * 128 : (b + 1) * 128])
                    ps = psw.tile([128, HID], F32, tag="psw")
                    nc.tensor.matmul(ps[:], lhsT=xt[:], rhs=w1_t[:], start=True, stop=True)
                    tb = ep.tile([128, HID], BF16, tag="tb")
                    nc.scalar.activation(tb[:], ps[:], AF.Copy, scale=dinv_t[:, b : b + 1])
                    nc.sync.dma_start(bnc[0][b * 128 : (b + 1) * 128, :], tb[:])
            with nc.named_scope("ag0"):
                nc.gpsimd.collective_compute(
                    "AllGather", OP.bypass, replica_groups=groups,
                    ins=[bnc[0].ap().opt()], outs=[tab[0].ap().opt()],
                )

            # ---- layers ----
            pool_ps = None
            for k in range(3):
                # pair views of the bf16 gather table: each 256B descriptor
                # fetches 2 adjacent rows; class r = 2*rp + h, rp picks the
                # pair within a row-quad, h the row within the pair.
                tview = tab[k].ap().rearrange("(a b) d -> a (b d)", b=R)
                rviews = [tview[:, (r // 2) * 2 * HID : (r // 2 + 1) * 2 * HID] for r in range(R)]
                if k == 2:
                    pool_ps = []
                    for gb in range(GB):
                        pps = psg.tile([128, HID], F32, tag=f"psg{gb}")
                        pool_ps.append(pps)

                sid, _ = nc.enter_named_scope(f"L{k}", notify=False)
                if k < 2:
                    hall = hap.tile([128, BLOCKS * HID], F32, tag="hall")
                else:
                    h3all = hap.tile([128, BLOCKS * HID], BF16, tag="h3all")
                for si, sw in enumerate(t.sws):
                    icw = sum(Lsr[si][r] // 16 for r in range(R))
                    ixt = ixp.tile([128, max(ix_max(t), 16)], I16, tag="ixt")
                    nc.sync.dma_start(ixt[:, :icw], gidx_d[:, icol[si][0] : icol[si][0] + icw])
                    dw = int(chunks[sw, :].sum())
                    ohl = ohp.tile([128, max(dl_max(t), 1) * 128], BF16, tag="ohl")
                    d0 = int(dcol[sw[0], 0])
                    nc.sync.dma_start(ohl[:, : dw * 128], ohd[:, d0 * 128 : (d0 + dw) * 128])

                    gts = [None] * R
                    for r in (1, 2, 3, 0):  # queue 0 runs sync on Pool: dispatch last
                        L = Lsr[si][r]
                        cols = L // 128
                        gt = mp.tile([128, max(m_max(t), 1), 2 * HID], BF16, tag=f"m{r}", name=f"gt{r}")
                        if L:
                            a0 = icol[si][r] - icol[si][0]
                            nc.gpsimd.dma_gather(
                                gt[:, :cols, :], rviews[r], ixt[:, a0 : a0 + L // 16],
                                L, L, 2 * HID, elem_step=R * HID,
                                single_packet=SINGLE_PACKET, queue_num=r,
                            )
                        gts[r] = gt

                    for b in sw:
                        ps = psb.tile([128, HID], F32, tag="psb")
                        nch = int(chunks[b, :].sum())
                        done = 0
                        for r in range(R):
                            ch = int(chunks[b, r])
                            if ch == 0:
                                continue
                            c0 = int(dcol[b, r]) - int(dcol[sw[0], 0])
                            hoff = (r % 2) * HID
                            for j in range(ch):
                                nc.tensor.matmul(
                                    ps[:],
                                    lhsT=ohl[:, (c0 + j) * 128 : (c0 + j + 1) * 128],
                                    rhs=gts[r][:, mcol[b, r] + j, hoff : hoff + HID],
                                    start=(done == 0),
                                    stop=(done == nch - 1),
                                )
                                done += 1

                        # ---- block epilogue: h = dinv*ps + (dinv*town + bias) ----
                        townb = ep.tile([128, HID], BF16, tag="townb")
                        nc.sync.dma_start(townb[:], bnc[k][b * 128 : (b + 1) * 128, :])
                        pre = ep.tile([128, HID], F32, tag="pre")
                        nc.scalar.activation(pre[:], townb[:], AF.Copy, scale=dinv_t[:, b : b + 1])
                        pre2 = ep.tile([128, HID], F32, tag="pre2")
                        nc.vector.tensor_add(pre2[:], pre[:], bt[k][:])
                        t1 = ep.tile([128, HID], F32, tag="t1")
                        nc.scalar.activation(t1[:], ps[:], AF.Copy, scale=dinv_t[:, b : b + 1])
                        hsl = slice(b * HID, (b + 1) * HID)
                        hp = ep.tile([128, HID], F32, tag="hp")
                        nc.vector.tensor_add(hp[:], t1[:], pre2[:])
                        if k < 2:
                            nc.scalar.activation(hall[:, hsl], hp[:], AF.Relu)
                        else:
                            nc.scalar.copy(h3all[:, hsl], hp[:])

                # ---- deferred PE phase: next-layer projection / pooling ----
                if k < 2:
                    for b in range(BLOCKS):
                        pt = pst.tile([128, 128], F32, tag="pst")
                        nc.tensor.transpose(pt[:HID, :], hall[:, b * HID : (b + 1) * HID], ident[:])
                        hT = ep.tile([HID, 128], F32, tag="hT")
                        nc.scalar.copy(hT[:], pt[:HID, :])
                        ps2 = psw.tile([128, HID], F32, tag="psw")
                        nc.tensor.matmul(ps2[:], lhsT=hT[:], rhs=wnext[k][:], start=True, stop=True)
                        tn = ep.tile([128, HID], BF16, tag="tb")
                        nc.scalar.activation(tn[:], ps2[:], AF.Copy, scale=dinv_t[:, b : b + 1])
                        nc.sync.dma_start(bnc[k + 1][b * 128 : (b + 1) * 128, :], tn[:])
                else:
                    for b in range(BLOCKS):
                        ohgl = ohp.tile([128, GB * 128], BF16, tag="ohg")
                        nc.sync.dma_start(ohgl[:], ohg_d[:, b * GB * 128 : (b + 1) * GB * 128])
                        for gb in range(GB):
                            gp = min(128, NG - gb * 128)
                            nc.tensor.matmul(
                                pool_ps[gb][:gp, :],
                                lhsT=ohgl[:, gb * 128 : gb * 128 + gp],
                                rhs=h3all[:, b * HID : (b + 1) * HID],
                                start=(b == 0),
                                stop=(b == BLOCKS - 1),
                            )
                nc.leave_named_scope(f"L{k}", sid, notify=False)
                if k < 2:
                    with nc.named_scope(f"ag{k+1}"):
                        nc.gpsimd.collective_compute(
                            "AllGather", OP.bypass, replica_groups=groups,
                            ins=[bnc[k + 1].ap().opt()], outs=[tab[k + 1].ap().opt()],
                        )

            # ---- pooling + final linear ----
            tid, _ = nc.enter_named_scope("tail", notify=False)
            for gb in range(GB):
                gp = min(128, NG - gb * 128)
                cpt = ep.tile([128, HID], F32, tag="t1")
                if gp < 128:
                    nc.any.memset(cpt[:], 0.0)
                nc.vector.tensor_copy(out=cpt[:gp, :], in_=pool_ps[gb][:gp, :])
                nc.sync.dma_start(pool_loc[gb * 128 : (gb + 1) * 128, :], cpt[:])
            nc.gpsimd.collective_compute(
                "AllReduce", OP.add, replica_groups=groups,
                ins=[pool_loc.ap().opt()], outs=[pool_sum.ap().opt()],
            )
            mx_t = ep.tile([128, GB], F32, tag="mx")
            nc.vector.tensor_scalar(mx_t[:], cnt_t[:], 1.0, None, OP.max)
            inv_t = ep.tile([128, GB], F32, tag="inv")
            nc.vector.reciprocal(inv_t[:], mx_t[:])
            for gb in range(GB):
                gp = min(128, NG - gb * 128)
                sm = ep.tile([128, HID], F32, tag="t1")
                nc.sync.dma_start(sm[:], pool_sum[gb * 128 : (gb + 1) * 128, :])
                mean = ep.tile([128, HID], F32, tag="h")
                nc.vector.tensor_scalar(mean[:], sm[:], inv_t[:, gb : gb + 1], None, OP.mult)
                pt = pst.tile([128, 128], F32, tag="pst")
                nc.tensor.transpose(pt[:HID, :], mean[:], ident[:])
                mT = ep.tile([HID, 128], F32, tag="hT")
                nc.scalar.copy(mT[:], pt[:HID, :])
                psf = psw.tile([128, NCLS], F32, tag="psw")
                nc.tensor.matmul(psf[:gp, :], lhsT=mT[:, :gp], rhs=wl_t[:], start=True, stop=True)
                of = ep.tile([128, NCLS], F32, tag="of")
                nc.vector.tensor_tensor(out=of[:gp, :], in0=psf[:gp, :], in1=blt_t[:gp, :], op=OP.add)
                nc.sync.dma_start(out_d[gb * 128 : gb * 128 + gp, :], of[:gp, :])
            nc.leave_named_scope("tail", tid, notify=False)

    nc.compile()
    return nc


def ix_max(t):
    return max(sum(t.Lsr[si][r] // 16 for r in range(R)) for si in range(len(t.sws)))


def dl_max(t):
    return max(int(t.chunks[sw, :].sum()) for sw in t.sws)


def m_max(t):
    return max(max(t.Lsr[si][r] // 128 for r in range(R)) for si in range(len(t.sws)))


def oh_max(t):
    return int(t.chunks.max())


# --------------------------------------------------------------------------
# Entry points
# --------------------------------------------------------------------------

def run_on_hw(inputs, cfg, trace=None):
    from concourse.bass_utils import run_bass_kernel_spmd

    if trace is None:
        trace = os.environ.get("GCN_TRACE", "0") == "1"
    t = preprocess(np.asarray(inputs["x"]), np.asarray(inputs["edge_index"]),
                   np.asarray(inputs["batch"]), cfg)
    in_maps = make_in_maps(
        t, *(np.asarray(inputs[k]) for k in
             ("W1", "b1", "W2", "b2", "W3", "b3", "Wlin", "blin")))
    nc = build_program(t)
    res = run_bass_kernel_spmd(nc, in_maps, core_ids=list(range(cfg["n_cores"])), trace=trace)
    run_on_hw.last = res
    return res.results[0]["out"].astype(np.float32)


def kernel(**inputs) -> np.ndarray:
    return run_on_hw(inputs, CFG_FULL)
